# revision 1
# baseline (speedup 1.0000x reference)
"""AttentionBlock kernel for Trainium2 (Bass/Tile), data-parallel over batch.

Reference computation (per batch b of 8, N = H*W = 4096, C = 256):
    q = x @ wq + bq ; k = x @ wk + bk ; v = x @ wv + bv          [N, C]
    s = (q @ k^T) / sqrt(C)                                      [N, N]
    a = softmax(s, axis=-1)
    o = a @ v                                                    [N, C]
    out = x + o @ wp + bp                                        [N, C]

Sharding: one batch per NeuronCore (8 batches, 8 cores), no collectives.

Per-core layout strategy ("S^T layout" — no attention transposes):
  - x is loaded naturally [n, c] and PE-transposed once to xT [c, n].
  - qT, kT [c, n] computed with weights as stationary operands.
  - v [n, c] computed naturally (xT slices stationary).
  - For each query block of 512 columns:
      for each key chunk m (32 chunks of 128 rows):
        sT[m-chunk]   = kT-slice.T @ qT-block      (PSUM [128k, 512q])
        eT = exp(sT / 16)                          (ACT, PSUM->SBUF)
        rawT[c-chunk] += v-slice.T @ eT            (PSUM [128c, 512q], accum)
        colsum += eT                               (DVE, SBUF accum)
      denom[q-sub] = colsum-slice.T @ ones         (partition reduce, [128q, 1])
      recip = 1/denom                              (DVE)
      proj[q-sub] = rawT-slice.T @ wp              (natural [128q, 256c])
      out = x + proj * recip + (bp + bv @ wp)      (DVE epilogue)
  The softmax denominator division is deferred: it commutes with the wp
  contraction because it is a per-query scaling. bv also commutes through
  (attention rows sum to 1), folded into an effective output bias.

Big matmul operands (xT, qT, kT, v, e, rawT, weights) are BF16: same PE
column rate as float32r but faster issue (216 vs 227 ns per 512-col matmul,
FWL weight loads) and half the SBUF read energy. The energy matters:
sustained full-rate activity across all engines trips the chip's P0 power
throttle (everything downclocks 2.4->2.0 GHz for the rest of the kernel,
and the state persists for minutes across runs). PSUM accumulation stays
fp32; end-to-end rel err 7.6e-3 vs the 2e-2 gate.

Schedule notes (hardware-measured; ~328us on a cool chip):
  - Phase 1 (xT transposes, q/k projections, v) is one merged per-nb
    pipeline (PE/ACT/DVE overlap; 85us -> ~37us vs serial phases), with the
    first x rows prefetched ahead of the serial weight DMAs.
  - Static PSUM pools for the whole kernel; a mid-kernel pool swap costs a
    ~3.5us drain barrier at the first attention block.
  - The attention schedule sits AT the P0 power threshold: removing its
    remaining pipeline stalls (deeper st buffering, denser epilogue)
    measured as a net loss every time (clock drops 20%). Check matmul
    issue deltas in the trace before trusting any local speedup
    (~216ns = full clock, ~260ns = throttled).
"""

import numpy as np

import concourse.mybir as mybir
import concourse.tile as tile
from concourse import bacc
from concourse import bass_utils
from concourse.masks import make_identity

# Problem shape (hardcoded per contract).
B, H, W, C = 8, 64, 64, 256
N = H * W  # 4096
P = 128
C2 = C // P  # 2 chunks of input/output channels
NK = N // P  # 32 key chunks
QB = 512  # query block width (free dim of S^T matmuls)
NQB = N // QB  # 8 query blocks
QSUB = QB // P  # 4 query sub-blocks of 128 per block
SCALE = float(C) ** -0.5  # 1/16

F32 = mybir.dt.float32
F32R = mybir.dt.float32r
BF16 = mybir.dt.bfloat16
AF = mybir.ActivationFunctionType

_CACHED_NC = None


def _build(repeat=1, rep_xt=1, rep_qkv=1, rep_attn=1):
    nc = bacc.Bacc("TRN2", target_bir_lowering=False, debug=False)

    x_d = nc.dram_tensor("x", [N, C], F32, kind="ExternalInput").ap()
    w_d = {
        name: nc.dram_tensor(name, [C, C], F32, kind="ExternalInput").ap()
        for name in ("wq", "wk", "wv", "wp")
    }
    b_d = {
        name: nc.dram_tensor(name, [C], F32, kind="ExternalInput").ap()
        for name in ("bq", "bk", "bv", "bp")
    }
    out_d = nc.dram_tensor("out", [N, C], F32, kind="ExternalOutput").ap()

    with tile.TileContext(nc) as tc:
        for _ in range(repeat):
            _emit(nc, tc, x_d, w_d, b_d, out_d, rep_xt, rep_qkv, rep_attn)
    nc.compile()
    return nc


def _emit(nc, tc, x_d, w_d, b_d, out_d, rep_xt=1, rep_qkv=1, rep_attn=1):
    import contextlib

    ctx = contextlib.ExitStack()
    with ctx:
        consts = ctx.enter_context(tc.tile_pool(name="consts", bufs=1))
        big = ctx.enter_context(tc.tile_pool(name="big", bufs=1))
        xload = ctx.enter_context(tc.tile_pool(name="xload", bufs=4))
        exp_pool = ctx.enter_context(tc.tile_pool(name="exp", bufs=4))
        sums = ctx.enter_context(tc.tile_pool(name="sums", bufs=2))
        rawsb = ctx.enter_context(tc.tile_pool(name="rawsb", bufs=2))
        epil = ctx.enter_context(tc.tile_pool(name="epil", bufs=4))

        # Static PSUM pools, 8 banks: st 2 (qkT in phase 1, scores in
        # attention) + raw 1 + proj 2 (v outs, wp proj) + misc 2 (transposes,
        # denominators). A mid-kernel pool swap (scoped phase pools) costs a
        # ~3.5us drain barrier at the first attention block - measured worse.
        ps_st = ctx.enter_context(tc.tile_pool(name="ps_st", bufs=2, space="PSUM"))
        ps_raw = ctx.enter_context(tc.tile_pool(name="ps_raw", bufs=1, space="PSUM"))
        ps_proj = ctx.enter_context(tc.tile_pool(name="ps_proj", bufs=2, space="PSUM"))
        ps_misc = ctx.enter_context(tc.tile_pool(name="ps_misc", bufs=2, space="PSUM"))

        # ---- constants -------------------------------------------------
        identity = consts.tile([P, P], F32)
        make_identity(nc, identity[:])

        # Prefetch the first query-block's x rows ahead of the (serial)
        # weight DMAs: the first transposes only need x + identity.
        x_pre = {}
        for h in range(2):
            xt = xload.tile([P, 2, C], F32, tag="x_in")
            n0 = h * 2 * P
            nc.sync.dma_start(
                xt[:], x_d[n0 : n0 + 2 * P, :].rearrange("(t p) c -> p t c", p=P)
            )
            x_pre[h] = xt

        # Tiny matmuls (denominator reduce, bias prep) run in plain fp32:
        # fp32r has ISA restrictions at small moving dims (N=1 is invalid).
        ones_col = consts.tile([P, 1], F32)
        nc.vector.memset(ones_col[:], 1.0)

        # Weights: [C, C] -> [P, C2, C] (ci = c2*128 + p on partitions).
        w_sb = {}
        for name in ("wq", "wk", "wv", "wp"):
            w_stage = consts.tile([P, C2, C], F32, tag=f"ws_{name}", name=f"ws_{name}")
            nc.sync.dma_start(
                w_stage[:], w_d[name].rearrange("(c2 p) co -> p c2 co", p=P)
            )
            w_sb[name] = consts.tile([P, C2, C], BF16, tag=f"w_{name}", name=f"w_{name}")
            nc.vector.tensor_copy(w_sb[name][:], w_stage[:])
        # Plain-fp32 copy of wp for the (tiny) bias-prep matmul.
        wp_f32 = consts.tile([P, C2, C], F32)
        nc.sync.dma_start(wp_f32[:], w_d["wp"].rearrange("(c2 p) co -> p c2 co", p=P))
        # bq, bk as per-partition scalars in the [co] layout: [P, C2].
        bqk_sb = {}
        for name in ("bq", "bk"):
            bqk_sb[name] = consts.tile([P, C2], F32, tag=f"b_{name}", name=f"b_{name}")
            nc.sync.dma_start(
                bqk_sb[name][:], b_d[name].rearrange("(c2 p) -> p c2", p=P)
            )
        # bv, bp as [1, C] rows (plain fp32 — the bias prep matmuls are tiny).
        bv_row = consts.tile([1, C], F32)
        bp_row = consts.tile([1, C], F32)
        nc.sync.dma_start(bv_row[:], b_d["bv"][None, :])
        nc.sync.dma_start(bp_row[:], b_d["bp"][None, :])

        # bp_eff[co] = bp[co] + sum_c bv[c] wp[c, co]; broadcast to [P, C].
        # Transpose bv_row to a column via matmul (K=1): bv_col = bv_row.T.
        bv_colps = ps_misc.tile([P, C2, 1], F32, tag="misc")
        for c2 in range(C2):
            # [1, 128] slice -> [128, 1]
            nc.tensor.matmul(
                bv_colps[:, c2],
                bv_row[:, c2 * P : (c2 + 1) * P],
                ones_col[:1],
                start=True,
                stop=True,
            )
        bv_col = consts.tile([P, C2, 1], F32)
        nc.vector.tensor_copy(bv_col[:], bv_colps[:])
        # bvwp[1, C] = sum_c2 bv_col[:, c2].T @ wp[:, c2, :]
        bvwp_ps = ps_misc.tile([1, C], F32, tag="misc")
        for c2 in range(C2):
            nc.tensor.matmul(
                bvwp_ps[:],
                bv_col[:, c2],
                wp_f32[:, c2, :],
                start=(c2 == 0),
                stop=(c2 == C2 - 1),
            )
        bp_eff_row = consts.tile([1, C], F32)
        nc.vector.tensor_add(bp_eff_row[:], bvwp_ps[:], bp_row[:])
        # Broadcast to all partitions: ones_col @ bp_eff_row.
        ones_row = consts.tile([1, P], F32)
        nc.vector.memset(ones_row[:], 1.0)
        bpb_ps = ps_misc.tile([P, C], F32, tag="misc")
        nc.tensor.matmul(bpb_ps[:], ones_row[:], bp_eff_row[:], start=True, stop=True)
        bp_bcast = consts.tile([P, C], F32)
        nc.vector.tensor_copy(bp_bcast[:], bpb_ps[:])

        # ---- merged phase 1: xT + qT/kT + v, interleaved per nb ------
        # Baseline ran xT (PE idle, copy-bound), then qkT (ACT-bound), then
        # v (DVE-bound) serially: 85us with every engine <40% busy. Merged,
        # the per-nb slices keep PE/ACT/DVE all fed (~3us per nb).
        xT = big.tile([P, C2, N], BF16, tag="xT")
        qT = big.tile([P, C2, N], BF16, tag="qT")
        kT = big.tile([P, C2, N], BF16, tag="kT")
        v_sb = big.tile([P, NK, C], BF16, tag="v")
        def stage_a(nb):
            # x rows for this 512-query block: 2 DMAs of [256, 256], then
            # transpose into xT (copies alternate ACT/DVE).
            nks = range(4 * nb, 4 * nb + 4)
            if nb == 0:
                x_tiles = x_pre
            else:
                x_tiles = {}
                for h in range(2):
                    xt = xload.tile([P, 2, C], F32, tag="x_in")
                    n0 = nb * QB + h * 2 * P
                    nc.sync.dma_start(
                        xt[:],
                        x_d[n0 : n0 + 2 * P, :].rearrange("(t p) c -> p t c", p=P),
                    )
                    x_tiles[h] = xt
            for j, nk in enumerate(nks):
                x_tile = x_tiles[j // 2][:, j % 2]
                for c2 in range(C2):
                    tps = ps_misc.tile([P, P], F32, tag="misc")
                    nc.tensor.transpose(
                        tps[:], x_tile[:, c2 * P : (c2 + 1) * P], identity[:]
                    )
                    dst_ap = xT[:, c2, nk * P : (nk + 1) * P]
                    if c2 == 0:
                        nc.scalar.copy(dst_ap, tps[:])
                    else:
                        nc.vector.tensor_copy(dst_ap, tps[:])

        def stage_b(nb):
            # v for the 4 nk chunks (stationary xT slices from stage_a(nb))
            for j, nk in enumerate(range(4 * nb, 4 * nb + 4)):
                pst = ps_proj.tile([P, C], F32, tag="mm_out")
                for ci2 in range(C2):
                    nc.tensor.matmul(
                        pst[:],
                        xT[:, ci2, nk * P : (nk + 1) * P],
                        w_sb["wv"][:, ci2, :],
                        start=(ci2 == 0),
                        stop=(ci2 == C2 - 1),
                    )
                # bv enters through bp_eff instead (attn rows sum to 1), so
                # v is the *raw* x@wv here.
                if j % 2 == 0:
                    nc.vector.tensor_copy(v_sb[:, nk, :], pst[:])
                else:
                    nc.scalar.copy(v_sb[:, nk, :], pst[:])
            # qT, kT blocks for this nb; bias-copies alternate ACT/DVE
            for dst, wname, bname, eng in (
                (qT, "wq", "bq", "act"),
                (kT, "wk", "bk", "dve"),
            ):
                for co2 in range(C2):
                    pst = ps_st.tile([P, QB], F32, tag="st")
                    for ci2 in range(C2):
                        nc.tensor.matmul(
                            pst[:],
                            w_sb[wname][:, ci2, co2 * P : (co2 + 1) * P],
                            xT[:, ci2, nb * QB : (nb + 1) * QB],
                            start=(ci2 == 0),
                            stop=(ci2 == C2 - 1),
                        )
                    dst_ap = dst[:, co2, nb * QB : (nb + 1) * QB]
                    bias_ap = bqk_sb[bname][:, co2 : co2 + 1]
                    if eng == "act":
                        nc.scalar.activation(dst_ap, pst[:], AF.Identity, bias=bias_ap)
                    else:
                        nc.vector.tensor_scalar_add(dst_ap, pst[:], bias_ap)

        # Sequential per-nb rhythm: a tighter software pipeline measured
        # FASTER locally but tripped the chip's P0 power throttle (everything
        # downclocks 2.4->2.0 GHz for the rest of the kernel) - net loss.
        for _ in range(rep_xt):
            for nb in range(NQB):
                stage_a(nb)
                stage_b(nb)

        # ---- attention over query blocks ------------------------------
        for _ in range(rep_attn):
         for qb in range(NQB):
            qslice = slice(qb * QB, (qb + 1) * QB)
            rawT_ps = ps_raw.tile([P, C2, QB], F32, tag="rawT")
            colsum = sums.tile([P, QB], F32, tag="colsum")

            for mk in range(NK):
                st_ps = ps_st.tile([P, QB], F32, tag="st")
                for ci2 in range(C2):
                    nc.tensor.matmul(
                        st_ps[:],
                        kT[:, ci2, mk * P : (mk + 1) * P],
                        qT[:, ci2, qslice],
                        start=(ci2 == 0),
                        stop=(ci2 == C2 - 1),
                    )
                e_t = exp_pool.tile([P, QB], BF16, tag="eT")
                nc.scalar.activation(e_t[:], st_ps[:], AF.Exp, scale=SCALE)
                # accumulate raw output (transposed)
                for c2 in range(C2):
                    nc.tensor.matmul(
                        rawT_ps[:, c2],
                        v_sb[:, mk, c2 * P : (c2 + 1) * P],
                        e_t[:],
                        start=(mk == 0),
                        stop=(mk == NK - 1),
                    )
                # accumulate softmax denominators
                if mk == 0:
                    nc.vector.tensor_copy(colsum[:], e_t[:])
                else:
                    nc.vector.tensor_add(colsum[:], colsum[:], e_t[:])

            # copy rawT to SBUF (fp32r: feeds the proj matmul)
            rawT_sb = rawsb.tile([P, C2, QB], BF16, tag="rawT_sb")
            nc.scalar.copy(rawT_sb[:, 0], rawT_ps[:, 0])
            nc.vector.tensor_copy(rawT_sb[:, 1], rawT_ps[:, 1])

            # denominators: [128q, 1] per q-sub via ones reduction (plain
            # fp32 matmul — N=1 is invalid for fp32r, and cost is trivial)
            den_ps = ps_misc.tile([P, QSUB], F32, tag="misc")
            for qs in range(QSUB):
                nc.tensor.matmul(
                    den_ps[:, qs : qs + 1],
                    colsum[:, qs * P : (qs + 1) * P],
                    ones_col[:],
                    start=True,
                    stop=True,
                )
            recip = sums.tile([P, QSUB], F32, tag="recip")
            nc.vector.reciprocal(recip[:], den_ps[:])

            # proj + epilogue per q-sub
            for qs in range(QSUB):
                pj_ps = ps_proj.tile([P, C], F32, tag="mm_out")
                for c2 in range(C2):
                    nc.tensor.matmul(
                        pj_ps[:],
                        rawT_sb[:, c2, qs * P : (qs + 1) * P],
                        w_sb["wp"][:, c2, :],
                        start=(c2 == 0),
                        stop=(c2 == C2 - 1),
                    )
                n0 = qb * QB + qs * P
                x_res = epil.tile([P, C], F32, tag="x_res")
                nc.sync.dma_start(x_res[:], x_d[n0 : n0 + P, :])
                o_t = epil.tile([P, C], F32, tag="o_t")
                # o = proj * recip[q]  (per-partition scalar)
                nc.vector.tensor_scalar_mul(o_t[:], pj_ps[:], recip[:, qs : qs + 1])
                # o += bp_eff (broadcast row)
                nc.vector.tensor_add(o_t[:], o_t[:], bp_bcast[:])
                # o += x residual
                nc.vector.tensor_add(o_t[:], o_t[:], x_res[:])
                nc.sync.dma_start(out_d[n0 : n0 + P, :], o_t[:])


def kernel(**inputs):
    global _CACHED_NC
    if _CACHED_NC is None:
        _CACHED_NC = _build()
    nc = _CACHED_NC

    x = np.ascontiguousarray(inputs["x"], dtype=np.float32)  # [B, H, W, C]
    shared = {
        name: np.ascontiguousarray(inputs[name], dtype=np.float32)
        for name in ("wq", "bq", "wk", "bk", "wv", "bv", "wp", "bp")
    }
    in_maps = [
        {"x": x[b].reshape(N, C), **shared} for b in range(B)
    ]
    res = bass_utils.run_bass_kernel_spmd(nc, in_maps, core_ids=list(range(B)))
    out = np.stack([res.results[b]["out"] for b in range(B)], axis=0)
    return out.reshape(B, H, W, C)



# revision 4
# speedup vs baseline: 1.2727x; 1.2727x over previous
"""AttentionBlock kernel for Trainium2 (Bass/Tile), data-parallel over batch.

Reference computation (per batch b of 8, N = H*W = 4096, C = 256):
    q = x @ wq + bq ; k = x @ wk + bk ; v = x @ wv + bv          [N, C]
    s = (q @ k^T) / sqrt(C)                                      [N, N]
    a = softmax(s, axis=-1)
    o = a @ v                                                    [N, C]
    out = x + o @ wp + bp                                        [N, C]

Sharding: one batch per NeuronCore (8 batches, 8 cores), no collectives.

Per-core layout strategy ("S^T layout" - no attention transposes):
  - x is loaded naturally [n, c] and PE-transposed once to xT [c, n].
  - qT, kT [c, n] computed with weights as stationary operands.
  - v [n, c] computed naturally (xT slices stationary).
  - For each query block of 512 columns:
      for each key chunk m (32 chunks of 128 rows):
        sT[m-chunk]   = kT-slice.T @ qT-block      (PSUM [128k, 512q])
        eT = exp(sT / 16)                          (ACT, PSUM->SBUF)
        rawT[c-chunk] += v-slice.T @ eT            (PSUM [128c, 512q], accum)
        colsum += eT                               (DVE, SBUF accum)
      denom[q-sub] = colsum-slice.T @ ones         (partition reduce, [128q, 1])
      recip = 1/denom                              (DVE)
      proj[q-sub] = rawT-slice.T @ wp              (natural [128q, 256c])
      out = proj * recip + (x + bp + bv @ wp)      (ACT scale + DVE add)
  The softmax denominator division is deferred: it commutes with the wp
  contraction because it is a per-query scaling. bv also commutes through
  (attention rows sum to 1), folded into an effective output bias.

Big matmul operands (xT, qT, kT, v, e, rawT, weights) are BF16 (PSUM
accumulation stays fp32; end-to-end rel err ~7.6e-3 vs the 2e-2 gate).
FP8/DoubleRow was evaluated numerically and blows the error gate (6.6e-2
for fp8 q/k alone) - e4m3's 3 mantissa bits are ~32x coarser than bf16.

Schedule (v2): the chip spends most runs P0-power-throttled at 2.0 GHz
(512-col matmul issue gap 259 ns instead of 216; the state persists for
minutes across runs), so the schedule targets zero PE stalls rather than
power economy:
  - The raw (v.T @ eT) pair for key-chunk mk issues one slot BEHIND the
    score pair for mk+1, so the exp(mk) activation has a full slot
    (~1 us) to drain before its consumer issues. The baseline's in-slot
    ordering stalled the PE ~370 ns every other chunk (~45 us total).
  - Per-query-block boundary work (denominator reduce, reciprocal, wp
    proj, epilogue) drains one piece per slot into the next block's
    stream instead of as a serial clump.
  - Head: DMA issues cost ~750 ns each on their issuing engine, so they
    are split across the two HWDGE queues - x loads on Sync, weights and
    biases on ACT - instead of one serial Sync chain. The x transposes
    are the first PE work (~10.5 us vs 18 us).
  - Phase 1 runs stage_a (transposes) two nb ahead of stage_b (qkv
    projections) so the PE never waits on x DMAs or PSUM-copy drains.
  - All of x stays resident in SBUF (32 KB/partition) - the epilogue
    residual add reads it directly instead of re-fetching from DRAM.
"""

import numpy as np

import concourse.mybir as mybir
import concourse.tile as tile
from concourse import bacc
from concourse import bass_utils
from concourse.masks import make_identity

# Problem shape (hardcoded per contract).
B, H, W, C = 8, 64, 64, 256
N = H * W  # 4096
P = 128
C2 = C // P  # 2 chunks of input/output channels
NK = N // P  # 32 key chunks
QB = 512  # query block width (free dim of S^T matmuls)
NQB = N // QB  # 8 query blocks
QSUB = QB // P  # 4 query sub-blocks of 128 per block
SCALE = float(C) ** -0.5  # 1/16

F32 = mybir.dt.float32
BF16 = mybir.dt.bfloat16
AF = mybir.ActivationFunctionType

_CACHED_NC = None


def _build():
    nc = bacc.Bacc("TRN2", target_bir_lowering=False, debug=False)

    x_d = nc.dram_tensor("x", [N, C], F32, kind="ExternalInput").ap()
    w_d = {
        name: nc.dram_tensor(name, [C, C], F32, kind="ExternalInput").ap()
        for name in ("wq", "wk", "wv", "wp")
    }
    b_d = {
        name: nc.dram_tensor(name, [C], F32, kind="ExternalInput").ap()
        for name in ("bq", "bk", "bv", "bp")
    }
    out_d = nc.dram_tensor("out", [N, C], F32, kind="ExternalOutput").ap()

    with tile.TileContext(nc) as tc:
        _emit(nc, tc, x_d, w_d, b_d, out_d)
    nc.compile()
    return nc


def _emit(nc, tc, x_d, w_d, b_d, out_d):
    import contextlib

    ctx = contextlib.ExitStack()
    with ctx:
        consts = ctx.enter_context(tc.tile_pool(name="consts", bufs=1))
        big = ctx.enter_context(tc.tile_pool(name="big", bufs=1))
        # x stays resident for the whole kernel: 8 nb tiles of
        # [128, 4, 256] fp32 (epilogue residual reads them in place).
        xload = ctx.enter_context(tc.tile_pool(name="xload", bufs=8))
        exp_pool = ctx.enter_context(tc.tile_pool(name="exp", bufs=4))
        sums = ctx.enter_context(tc.tile_pool(name="sums", bufs=2))
        rawsb = ctx.enter_context(tc.tile_pool(name="rawsb", bufs=2))
        epil = ctx.enter_context(tc.tile_pool(name="epil", bufs=4))

        # Static PSUM pools, 8 banks: st 2 + raw 2 + proj(v/wp outs) 2 +
        # misc (transposes, denominators, bias prep) 2.
        ps_st = ctx.enter_context(tc.tile_pool(name="ps_st", bufs=2, space="PSUM"))
        ps_raw = ctx.enter_context(tc.tile_pool(name="ps_raw", bufs=1, space="PSUM"))
        ps_proj = ctx.enter_context(tc.tile_pool(name="ps_proj", bufs=2, space="PSUM"))
        ps_misc = ctx.enter_context(tc.tile_pool(name="ps_misc", bufs=2, space="PSUM"))

        # ---- constants + DMA issues -----------------------------------
        # DMA issues cost ~750 ns each on the issuing engine and are
        # strictly ordered per queue: x loads go on Sync, weights and
        # bias rows on ACT (both are HWDGE engines) so the chains overlap.
        identity = consts.tile([P, P], F32)
        make_identity(nc, identity[:])

        x_tiles = {}  # nb -> [P, 4, C] fp32 tile (rows nb*512 .. +511)
        for nb in range(NQB):
            xt = xload.tile([P, 4, C], F32, tag="x_in", name=f"x_in_{nb}")
            n0 = nb * QB
            nc.sync.dma_start(
                xt[:], x_d[n0 : n0 + QB, :].rearrange("(t p) c -> p t c", p=P)
            )
            x_tiles[nb] = xt

        ones_col = consts.tile([P, 1], F32)
        nc.vector.memset(ones_col[:], 1.0)
        ones_row = consts.tile([1, P], F32)
        nc.vector.memset(ones_row[:], 1.0)

        # Weights: [C, C] -> [P, C2, C] (ci = c2*128 + p on partitions),
        # issued on the ACT queue, wv first (stage_b(0) needs it first).
        w_stage = {}
        w_sb = {}
        for name in ("wv", "wq", "wk", "wp"):
            w_stage[name] = consts.tile(
                [P, C2, C], F32, tag=f"ws_{name}", name=f"ws_{name}"
            )
            nc.scalar.dma_start(
                w_stage[name][:], w_d[name].rearrange("(c2 p) co -> p c2 co", p=P)
            )
            w_sb[name] = consts.tile([P, C2, C], BF16, tag=f"w_{name}", name=f"w_{name}")
        # bq, bk as per-partition scalars [P, C2]; bv, bp as [1, C] rows.
        bqk_sb = {}
        for name in ("bq", "bk"):
            bqk_sb[name] = consts.tile([P, C2], F32, tag=f"b_{name}", name=f"b_{name}")
            nc.scalar.dma_start(
                bqk_sb[name][:], b_d[name].rearrange("(c2 p) -> p c2", p=P)
            )
        bv_row = consts.tile([1, C], F32)
        bp_row = consts.tile([1, C], F32)
        nc.scalar.dma_start(bv_row[:], b_d["bv"][None, :])
        nc.scalar.dma_start(bp_row[:], b_d["bp"][None, :])

        # ---- phase 1 stages -------------------------------------------
        xT = big.tile([P, C2, N], BF16, tag="xT")
        qT = big.tile([P, C2, N], BF16, tag="qT")
        kT = big.tile([P, C2, N], BF16, tag="kT")
        v_sb = big.tile([P, NK, C], BF16, tag="v")

        def stage_a(nb, dve_only=False):
            # transpose x rows for this 512-query block into xT
            # (copies out of the 2-deep misc PSUM ring; all-DVE for the
            # first two nb while ACT is still issuing weight DMAs).
            for j, nk in enumerate(range(4 * nb, 4 * nb + 4)):
                x_tile = x_tiles[nb][:, j]
                for c2 in range(C2):
                    tps = ps_misc.tile([P, P], F32, tag="misc")
                    nc.tensor.transpose(
                        tps[:], x_tile[:, c2 * P : (c2 + 1) * P], identity[:]
                    )
                    dst_ap = xT[:, c2, nk * P : (nk + 1) * P]
                    if dve_only or c2 == 1:
                        nc.vector.tensor_copy(dst_ap, tps[:])
                    else:
                        nc.scalar.copy(dst_ap, tps[:])

        def stage_b(nb):
            # v for the 4 nk chunks (stationary xT slices from stage_a(nb))
            for j, nk in enumerate(range(4 * nb, 4 * nb + 4)):
                pst = ps_proj.tile([P, C], F32, tag="mm_out")
                for ci2 in range(C2):
                    nc.tensor.matmul(
                        pst[:],
                        xT[:, ci2, nk * P : (nk + 1) * P],
                        w_sb["wv"][:, ci2, :],
                        start=(ci2 == 0),
                        stop=(ci2 == C2 - 1),
                    )
                # bv enters through bp_eff instead (attn rows sum to 1), so
                # v is the *raw* x@wv here.
                if j % 2 == 0:
                    nc.vector.tensor_copy(v_sb[:, nk, :], pst[:])
                else:
                    nc.scalar.copy(v_sb[:, nk, :], pst[:])
            # qT, kT blocks for this nb; bias-copies alternate ACT/DVE
            for dst, wname, bname, eng in (
                (qT, "wq", "bq", "act"),
                (kT, "wk", "bk", "dve"),
            ):
                for co2 in range(C2):
                    pst = ps_st.tile([P, QB], F32, tag="st")
                    for ci2 in range(C2):
                        nc.tensor.matmul(
                            pst[:],
                            w_sb[wname][:, ci2, co2 * P : (co2 + 1) * P],
                            xT[:, ci2, nb * QB : (nb + 1) * QB],
                            start=(ci2 == 0),
                            stop=(ci2 == C2 - 1),
                        )
                    dst_ap = dst[:, co2, nb * QB : (nb + 1) * QB]
                    bias_ap = bqk_sb[bname][:, co2 : co2 + 1]
                    if eng == "act":
                        nc.scalar.activation(dst_ap, pst[:], AF.Identity, bias=bias_ap)
                    else:
                        nc.vector.tensor_scalar_add(dst_ap, pst[:], bias_ap)

        bp_bcast = None

        def bias_prep():
            # bp_eff[co] = bp[co] + sum_c bv[c] wp[c, co]; broadcast [P, C].
            # First needed by the qb=0 epilogue precompute, ~60 us in.
            nonlocal bp_bcast
            bv_colps = ps_misc.tile([P, C2, 1], F32, tag="misc")
            for c2 in range(C2):
                nc.tensor.matmul(
                    bv_colps[:, c2],
                    bv_row[:, c2 * P : (c2 + 1) * P],
                    ones_col[:1],
                    start=True,
                    stop=True,
                )
            bv_col = consts.tile([P, C2, 1], F32)
            nc.vector.tensor_copy(bv_col[:], bv_colps[:])
            bvwp_ps = ps_misc.tile([1, C], F32, tag="misc")
            for c2 in range(C2):
                nc.tensor.matmul(
                    bvwp_ps[:],
                    bv_col[:, c2],
                    w_stage["wp"][:, c2, :],
                    start=(c2 == 0),
                    stop=(c2 == C2 - 1),
                )
            bp_eff_row = consts.tile([1, C], F32)
            nc.vector.tensor_add(bp_eff_row[:], bvwp_ps[:], bp_row[:])
            bpb_ps = ps_misc.tile([P, C], F32, tag="misc")
            nc.tensor.matmul(
                bpb_ps[:], ones_row[:], bp_eff_row[:], start=True, stop=True
            )
            bp_bcast = consts.tile([P, C], F32)
            nc.vector.tensor_copy(bp_bcast[:], bpb_ps[:])

        # ---- phase 1: stage_a two nb ahead of stage_b -----------------
        stage_a(0, dve_only=True)
        stage_a(1, dve_only=True)
        # Weight casts emitted here (DVE program order: after the first
        # transposes' copies, which they'd otherwise head-of-line block).
        nc.vector.tensor_copy(w_sb["wv"][:], w_stage["wv"][:])
        nc.vector.tensor_copy(w_sb["wq"][:], w_stage["wq"][:])
        nc.vector.tensor_copy(w_sb["wk"][:], w_stage["wk"][:])

        for nb in range(NQB):
            stage_b(nb)
            if nb + 2 < NQB:
                stage_a(nb + 2)
            if nb == 2:
                nc.vector.tensor_copy(w_sb["wp"][:], w_stage["wp"][:])
                bias_prep()

        # ---- attention: flat software-pipelined stream ----------------
        # Slot t emits: st-pair(t), raw-pair(t-1), one piece of boundary
        # work. exp(t) goes on ACT right after its st-pair; colsum(t-1)
        # on DVE right after its raw-pair.
        state = {}  # qb -> dict(rawT_ps, colsum, e, recip, rawT_sb, xbp)

        def st_pair(qb, mk):
            st_ps = ps_st.tile([P, QB], F32, tag="st")
            for ci2 in range(C2):
                nc.tensor.matmul(
                    st_ps[:],
                    kT[:, ci2, mk * P : (mk + 1) * P],
                    qT[:, ci2, qb * QB : (qb + 1) * QB],
                    start=(ci2 == 0),
                    stop=(ci2 == C2 - 1),
                )
            e_t = exp_pool.tile([P, QB], BF16, tag="eT")
            nc.scalar.activation(e_t[:], st_ps[:], AF.Exp, scale=SCALE)
            return e_t

        def raw_pair(qb, mk, e_t):
            st_ = state.setdefault(qb, {})
            if mk == 0:
                st_["rawT_ps"] = ps_raw.tile(
                    [P, C2, QB], F32, tag="rawT", name=f"rawT_{qb}"
                )
                st_["colsum"] = sums.tile(
                    [P, QB], F32, tag="colsum", name=f"colsum_{qb}"
                )
            rawT_ps = st_["rawT_ps"]
            colsum = st_["colsum"]
            for c2 in range(C2):
                nc.tensor.matmul(
                    rawT_ps[:, c2],
                    v_sb[:, mk, c2 * P : (c2 + 1) * P],
                    e_t[:],
                    start=(mk == 0),
                    stop=(mk == NK - 1),
                )
            if mk == 0:
                nc.vector.tensor_copy(colsum[:], e_t[:])
            else:
                nc.vector.tensor_add(colsum[:], colsum[:], e_t[:])
            if mk == NK - 1:
                # rawT -> SBUF bf16 for the proj matmuls (ACT and DVE
                # take one half each, in parallel).
                rawT_sb = rawsb.tile([P, C2, QB], BF16, tag="rawT_sb")
                nc.scalar.copy(rawT_sb[:, 0], rawT_ps[:, 0])
                nc.vector.tensor_copy(rawT_sb[:, 1], rawT_ps[:, 1])
                st_["rawT_sb"] = rawT_sb

        def extra_den(qb):
            st_ = state[qb]
            den_ps = ps_misc.tile([P, QSUB], F32, tag="misc")
            for qs in range(QSUB):
                nc.tensor.matmul(
                    den_ps[:, qs : qs + 1],
                    st_["colsum"][:, qs * P : (qs + 1) * P],
                    ones_col[:],
                    start=True,
                    stop=True,
                )
            recip = sums.tile([P, QSUB], F32, tag="recip")
            nc.vector.reciprocal(recip[:], den_ps[:])
            st_["recip"] = recip

        def extra_proj(qb, qs):
            st_ = state[qb]
            pj_ps = ps_proj.tile([P, C], F32, tag="mm_out")
            for c2 in range(C2):
                nc.tensor.matmul(
                    pj_ps[:],
                    st_["rawT_sb"][:, c2, qs * P : (qs + 1) * P],
                    w_sb["wp"][:, c2, :],
                    start=(c2 == 0),
                    stop=(c2 == C2 - 1),
                )
            n0 = qb * QB + qs * P
            # o1 = proj * recip[q] on ACT (per-partition scale AP)
            o_t = epil.tile([P, C], F32, tag="o_t")
            nc.scalar.activation(
                o_t[:], pj_ps[:], AF.Identity, scale=st_["recip"][:, qs : qs + 1]
            )
            # o = o1 + (x + bp_eff)  (xbp precomputed off-critical-path)
            nc.vector.tensor_add(o_t[:], o_t[:], st_["xbp"][qs][:])
            nc.sync.dma_start(out_d[n0 : n0 + P, :], o_t[:])
            if qs == QSUB - 1:
                del state[qb]

        def emit_xbp(qb, qs):
            # xbp = x + bp_bcast for sub-block qs of qb (DVE, early)
            st_ = state.setdefault(qb, {})
            xbp = epil.tile([P, C], F32, tag="xbp")
            nc.vector.tensor_add(xbp[:], x_tiles[qb][:, qs], bp_bcast[:])
            st_.setdefault("xbp", {})[qs] = xbp

        pend = None  # (qb, mk, e_t) whose raw-pair is still to issue
        extras = []  # deferred boundary closures, one per slot

        for qb in range(NQB):
            for mk in range(NK):
                e_t = st_pair(qb, mk)
                if pend is not None:
                    raw_pair(pend[0], pend[1], pend[2])
                pend = (qb, mk, e_t)
                if extras:
                    extras.pop(0)()
                elif 8 <= mk < 12:
                    emit_xbp(qb, mk - 8)
            # anything not yet drained from the previous boundary
            while extras:
                extras.pop(0)()
            extras = [lambda qb=qb: extra_den(qb)] + [
                lambda qb=qb, qs=qs: extra_proj(qb, qs) for qs in range(QSUB)
            ]
        raw_pair(pend[0], pend[1], pend[2])
        while extras:
            extras.pop(0)()


def kernel(**inputs):
    global _CACHED_NC
    if _CACHED_NC is None:
        _CACHED_NC = _build()
    nc = _CACHED_NC

    x = np.ascontiguousarray(inputs["x"], dtype=np.float32)  # [B, H, W, C]
    shared = {
        name: np.ascontiguousarray(inputs[name], dtype=np.float32)
        for name in ("wq", "bq", "wk", "bk", "wv", "bv", "wp", "bp")
    }
    in_maps = [
        {"x": x[b].reshape(N, C), **shared} for b in range(B)
    ]
    res = bass_utils.run_bass_kernel_spmd(nc, in_maps, core_ids=list(range(B)))
    out = np.stack([res.results[b]["out"] for b in range(B)], axis=0)
    return out.reshape(B, H, W, C)
